# revision 10
# baseline (speedup 1.0000x reference)
"""GuidedAttentionLoss on Trainium2 — 8 NeuronCores, batch-parallel.

loss = mean(attention_weights * mask), mask[b,i,j] =
    (i < out_len_b) & (j < in_len_b) ? exp(-(j - floor(i/out*in))^2 / (2*0.4^2)) : 0

Strategy: shard B=64 across 8 cores (8 batches each). Each core streams its
25.6MB attention shard through SBUF in [128, 400] row-tiles, generates the
mask on the fly and does a fused multiply+reduce. Per-(b,i) scalars
(ideal_j etc.) are precomputed on host (tiny [B,T] work) and fed as f32
tables; validity is folded into the exp argument so no per-element select
ops are needed:

  g = exp(-3.125 * (j - ideal)^2)  with  -3.125 == -1/(2*sigma^2) exactly
  row-invalid  -> ideal := -1e4           => exp arg <= -3.1e8 => g = 0
  col-invalid  -> j     := j + 1e4 (A) / arg += -3.125e7 (B)   => g = 0

Two per-tile mask pipelines, interleaved to balance engines:
  A: ACT Square(jc - ideal) ; ACT Exp(-3.125*d2)
  B: DVE/GPSIMD stt t = j*(-2*ideal) + (j^2 + colpen) ; ACT Exp(-3.125*t - 3.125*ideal^2)
Both end with DVE tensor_tensor_reduce (g*attn, sum) -> acc[:, tile].
Host sums the 8 [128,128] per-core accumulators for the global mean.
"""

import numpy as np

import concourse.bacc as bacc
import concourse.bass as bass  # noqa: F401  (AP types etc.)
import concourse.mybir as mybir
from concourse import tile
from concourse.bass_utils import run_bass_kernel_spmd

N_CORES = 8
B, T, E = 64, 2000, 400
B_LOC = B // N_CORES  # 8 batches per core
P = 128
NT = 16  # row-tiles per batch: 15 full + 1 tail overlapping (rows 1872..1999)
NTILES = B_LOC * NT  # 128 tiles per core
NEG_SCALE = -3.125  # -1/(2*sigma^2), exact in binary fp
F32 = mybir.dt.float32
AF = mybir.ActivationFunctionType
OP = mybir.AluOpType

_TILE_I0 = [min(ti * P, T - P) for ti in range(NT)]

# per-tile mask-gen engine: A = ACT Square+Exp, B = DVE stt + ACT Exp,
# C = GPSIMD stt + ACT Exp.  Tune mix from trace.
TYPE_PATTERN = (["A", "B"] * NT)[:NT]

_NC_CACHE = {}


def _build_nc(rep=1):
    nc = bacc.Bacc(None, target_bir_lowering=False)
    attn = nc.declare_dram_parameter("attn", [B_LOC, T, E], F32, isOutput=False)
    negideal_d = nc.declare_dram_parameter("negideal", [P, NTILES], F32, isOutput=False)
    m2i_d = nc.declare_dram_parameter("m2i", [P, NTILES], F32, isOutput=False)
    n3i2_d = nc.declare_dram_parameter("n3i2", [P, NTILES], F32, isOutput=False)
    inlen_d = nc.declare_dram_parameter("inlen", [P, B_LOC], F32, isOutput=False)
    acc_d = nc.declare_dram_parameter("acc", [P, NTILES], F32, isOutput=True)

    with tile.TileContext(nc) as tc:
        with (
            tc.tile_pool(name="const", bufs=1) as const_pool,
            tc.tile_pool(name="batch", bufs=2) as batch_pool,
            tc.tile_pool(name="attn", bufs=4) as attn_pool,
            tc.tile_pool(name="work", bufs=4) as work_pool,
            tc.tile_pool(name="g", bufs=4) as g_pool,
            tc.tile_pool(name="junk", bufs=4) as junk_pool,
        ):
            j_i32 = const_pool.tile([P, E], mybir.dt.int32, tag="j_i32")
            j_f32 = const_pool.tile([P, E], F32, tag="j_f32")
            j2 = const_pool.tile([P, E], F32, tag="j2")
            negideal = const_pool.tile([P, NTILES], F32, tag="negideal")
            m2i = const_pool.tile([P, NTILES], F32, tag="m2i")
            n3i2 = const_pool.tile([P, NTILES], F32, tag="n3i2")
            inlen = const_pool.tile([P, B_LOC], F32, tag="inlen")
            acc = const_pool.tile([P, NTILES], F32, tag="acc")

            nc.gpsimd.iota(j_i32[:], pattern=[[1, E]], base=0, channel_multiplier=0)
            nc.vector.tensor_copy(j_f32[:], j_i32[:])
            nc.vector.tensor_tensor(j2[:], j_f32[:], j_f32[:], OP.mult)
            nc.sync.dma_start(out=negideal[:], in_=negideal_d[:])
            nc.sync.dma_start(out=m2i[:], in_=m2i_d[:])
            nc.sync.dma_start(out=n3i2[:], in_=n3i2_d[:])
            nc.sync.dma_start(out=inlen[:], in_=inlen_d[:])

            for _r, lb in ((r, b) for r in range(rep) for b in range(B_LOC)):
                # col-invalid (j >= in_len) penalties, built once per batch
                cm = batch_pool.tile([P, E], F32, tag="cm")
                nc.vector.tensor_scalar(
                    cm[:], j_f32[:], inlen[:, lb : lb + 1], None, OP.is_ge
                )
                jc = batch_pool.tile([P, E], F32, tag="jc")  # j + 1e4*colinv
                nc.vector.scalar_tensor_tensor(
                    jc[:], cm[:], 1e4, j_f32[:], OP.mult, OP.add
                )
                vcol = batch_pool.tile([P, E], F32, tag="vcol")  # j^2 + 1e7*colinv
                nc.vector.scalar_tensor_tensor(
                    vcol[:], cm[:], 1e7, j2[:], OP.mult, OP.add
                )
                for ti in range(NT):
                    col = lb * NT + ti
                    i0 = _TILE_I0[ti]
                    at = attn_pool.tile([P, E], F32, tag="at")
                    nc.sync.dma_start(out=at[:], in_=attn[lb, i0 : i0 + P, :])
                    g = g_pool.tile([P, E], F32, tag="g")
                    ty = TYPE_PATTERN[ti]
                    if ty == "A":
                        d2 = work_pool.tile([P, E], F32, tag="d2")
                        nc.scalar.activation(
                            d2[:],
                            jc[:],
                            AF.Square,
                            bias=negideal[:, col : col + 1],
                            scale=1.0,
                        )
                        nc.scalar.activation(g[:], d2[:], AF.Exp, scale=NEG_SCALE)
                    else:
                        tt = work_pool.tile([P, E], F32, tag="d2")
                        eng = nc.vector if ty == "B" else nc.gpsimd
                        eng.scalar_tensor_tensor(
                            tt[:],
                            j_f32[:],
                            m2i[:, col : col + 1],
                            vcol[:],
                            OP.mult,
                            OP.add,
                        )
                        nc.scalar.activation(
                            g[:],
                            tt[:],
                            AF.Exp,
                            bias=n3i2[:, col : col + 1],
                            scale=NEG_SCALE,
                        )
                    jk = junk_pool.tile([P, E], F32, tag="jk")
                    # fused (g*attn) + row-sum; tensor_tensor_reduce dies at
                    # runtime in this environment, stt+accum_out is equivalent
                    nc.vector.scalar_tensor_tensor(
                        jk[:],
                        g[:],
                        1.0,
                        at[:],
                        OP.mult,
                        OP.mult,
                        accum_out=acc[:, col : col + 1],
                    )
            nc.sync.dma_start(out=acc_d[:], in_=acc[:])
    return nc


def _get_nc(rep=1):
    if rep not in _NC_CACHE:
        nc = _build_nc(rep)
        if not nc.is_finalized():
            nc.finalize()  # runs Bacc passes (wait splitting, reg alloc, ...)
        _NC_CACHE[rep] = nc
    return _NC_CACHE[rep]


def _make_tables(input_lengths, output_lengths, core):
    sl = slice(core * B_LOC, (core + 1) * B_LOC)
    in_len = np.asarray(input_lengths[sl], dtype=np.float32)
    out_len_i = np.asarray(output_lengths[sl], dtype=np.int64)
    safe_out = np.maximum(np.asarray(output_lengths[sl], dtype=np.float32), np.float32(1.0))
    negideal = np.empty((P, NTILES), np.float32)
    m2i = np.empty((P, NTILES), np.float32)
    n3i2 = np.empty((P, NTILES), np.float32)
    inlen = np.repeat(in_len[None, :], P, axis=0).astype(np.float32)
    p = np.arange(P, dtype=np.int64)
    for lb in range(B_LOC):
        for ti in range(NT):
            i0 = _TILE_I0[ti]
            i = i0 + p
            # replicate the reference's f32 arithmetic exactly
            i_f = i.astype(np.float32)
            ideal = np.floor((i_f / safe_out[lb]) * in_len[lb]).astype(np.float32)
            valid = (i < out_len_i[lb]) & (i >= ti * P)  # tail tile overlap dedup
            ideal_eff = np.where(valid, ideal, np.float32(-1e4)).astype(np.float32)
            col = lb * NT + ti
            negideal[:, col] = -ideal_eff
            m2i[:, col] = np.float32(-2.0) * ideal_eff
            n3i2[:, col] = (
                np.float64(-3.125) * ideal_eff.astype(np.float64) ** 2
            ).astype(np.float32)
    return {"negideal": negideal, "m2i": m2i, "n3i2": n3i2, "inlen": inlen}


def _run(attention_weights, input_lengths, output_lengths, **spmd_kwargs):
    attention_weights = np.ascontiguousarray(attention_weights, dtype=np.float32)
    in_maps = []
    for c in range(N_CORES):
        in_maps.append(
            {
                "attn": np.ascontiguousarray(
                    attention_weights[c * B_LOC : (c + 1) * B_LOC]
                ),
                **_make_tables(input_lengths, output_lengths, c),
            }
        )
    res = run_bass_kernel_spmd(_get_nc(), in_maps, list(range(N_CORES)), **spmd_kwargs)
    total = sum(float(r["acc"].sum(dtype=np.float64)) for r in res.results)
    return np.float32(total / float(B * T * E)), res


def kernel(attention_weights, input_lengths, output_lengths):
    out, _ = _run(attention_weights, input_lengths, output_lengths)
    return out
